# revision 37
# baseline (speedup 1.0000x reference)
"""Trainium2 Bass kernel for AttentionGuidedConv.

Reference semantics (B=C=96, L=8192, K=31, A=512):
    kernels = attention_weights @ proj_w.T + proj_b          # [96, 31]
    y[b, t, o] = sum_k x[b, t+k, o] * kernels[o, k]          # [96, 8162, 96]

Note the conv weight depends only on the channel index o (the depthwise
conv uses channel o's kernel for every batch element).

Strategy:
  - Shard batch dim B=96 across 8 cores (12 batches/core, contiguous HBM).
  - Per (batch, channel): depthwise conv as a banded-Toeplitz matmul on
    TensorE. Time axis is tiled in hops of 98 with a 128-deep window:
    chunk n covers outputs t = 98n + m (m in [0,98)), contraction over the
    128-row window x[98n + p].  Stationary = [128, 98] band matrix with
    band[p, m] = kern[o, p-m] for 0 <= p-m < 31; moving = x window
    [128, 84] (all 84 chunks of one batch, channel-strided AP).
  - 6 channels share one PSUM bank ([98, 6, 84] = 2016B/partition), so
    PSUM->SBUF copies move 504 elements per instruction (DVE/ACT split).
  - Band matrices are built host-side (pure weight layout; the tiny
    attention projection is 0.00006% of total FLOPs) and DMA'd once.
"""

import os

import numpy as np

import concourse.bass as bass
import concourse.bacc as bacc
import concourse.mybir as mybir
import concourse.tile as tile
from concourse.bass_utils import run_bass_kernel_spmd

F32 = mybir.dt.float32
_MM_DT_NAME = os.environ.get("KERNEL_MM_DT", "f16")
MM_DT = {
    "f32": mybir.dt.float32,
    "f32r": mybir.dt.float32r,   # FP22 matmul: 1 pass/col instead of fp32's 2
    "bf16": mybir.dt.bfloat16,   # halves input DMA bytes; ~3e-3 absmax-rel err
    "f16": mybir.dt.float16,     # halves input DMA bytes; ~3e-4 absmax-rel err
}[_MM_DT_NAME]
_OUT_DT_NAME = os.environ.get("KERNEL_OUT_DT", "f16")
OUT_DT = {
    "f32": mybir.dt.float32,
    "f16": mybir.dt.float16,     # halves output DMA bytes; +~5e-4 rounding
}[_OUT_DT_NAME]
SINGLE_PACKET = os.environ.get("KERNEL_SINGLE_PACKET", "0") == "1"
COPY_MODE = os.environ.get("KERNEL_COPY_MODE", "dve")  # split | dve
# DMA granularity: chunks per dma_start (0 = whole batch in one DMA).
# Smaller = address-sequential descriptors (DRAM row locality), more instrs.
IN_DMA_CHUNKS = int(os.environ.get("KERNEL_IN_DMA_CHUNKS", "0"))
OUT_DMA_CHUNKS = int(os.environ.get("KERNEL_OUT_DMA_CHUNKS", "0"))

B, L, C = 96, 8192, 96
K = 31
A = 512
N_CORES = 8
B_SHARD = B // N_CORES          # 12 batches per core

HOP = 98                        # outputs per chunk (98 + 31 - 1 <= 128)
WIN = 128                       # contraction window (partition dim)
L_OUT = L - K + 1               # 8162
N_CHUNKS = (L_OUT + HOP - 1) // HOP      # 84
N_FULL = L_OUT // HOP                    # 83 full chunks
LAST_START = L - WIN                     # 8064: last chunk window start
LAST_FRESH = L_OUT - N_FULL * HOP        # 28 fresh outputs in last chunk
LAST_FRESH_OFF = N_FULL * HOP - LAST_START   # 70: first fresh row of last chunk

B_BLK = 2                       # batches per block (one stationary load covers
                                # B_BLK*84 = 168 moving columns)
C_GRP = 3                       # channels per PSUM bank (3*168*4B = 2016 <= 2KB)
N_GRP = C // C_GRP              # 32 groups


def build_nc(b_shard: int = B_SHARD, mm_dt=MM_DT, out_dt=OUT_DT) -> bass.Bass:
    nc = bacc.Bacc(None, target_bir_lowering=False)
    x_d = nc.dram_tensor("x", [b_shard, L, C], mm_dt, kind="ExternalInput")
    bands_d = nc.dram_tensor("bands", [WIN, C * HOP], mm_dt, kind="ExternalInput")
    y_d = nc.dram_tensor("y", [b_shard, L_OUT, C], out_dt, kind="ExternalOutput")

    with tile.TileContext(nc) as tc:
        xh_bufs = int(os.environ.get("KERNEL_XH_BUFS", "4"))
        out_bufs = int(os.environ.get("KERNEL_OUT_BUFS", "2"))
        with (
            tc.tile_pool(name="const", bufs=1) as const_pool,
            tc.tile_pool(name="xh", bufs=xh_bufs) as xh_pool,
            tc.tile_pool(name="out", bufs=out_bufs) as out_pool,
            tc.tile_pool(name="psum", bufs=8, space="PSUM") as psum_pool,
        ):
            bands_sb = const_pool.tile([WIN, C, HOP], mm_dt)
            nc.scalar.dma_start(bands_sb[:, :, :], bands_d[:, :].rearrange("p (c m) -> p c m", c=C))

            assert b_shard % B_BLK == 0
            n_blk = b_shard // B_BLK

            def do_block(blk, b0, lo, hi, in_eng, out_eng, out_halves=1):
                """Process chunks [lo, hi) of batches [b0, b0+B_BLK)."""
                nch = hi - lo
                has_tail = hi == N_CHUNKS          # includes the 8064-window chunk
                nfull = nch - 1 if has_tail else nch
                xh = xh_pool.tile([WIN, B_BLK, nch, C], mm_dt, tag="xh")
                for s in range(B_BLK):
                    srcA = x_d[b0 + s, 0, :].copy()
                    srcA.ap = mybir.VecI64Pair(
                        [[C, WIN], [HOP * C, nfull], [1, C]]
                    )
                    srcA.offset = srcA.offset + lo * HOP * C
                    in_eng.dma_start(xh[:, s, 0:nfull, :], srcA,
                                     single_packet=SINGLE_PACKET)
                if has_tail:
                    srcB = x_d[b0, LAST_START, :].copy()
                    srcB.ap = mybir.VecI64Pair([[C, WIN], [L * C, B_BLK], [1, C]])
                    in_eng.dma_start(xh[:, :, nfull, :], srcB,
                                     single_packet=SINGLE_PACKET)

                # out tile split into halves: the store of a half can start as
                # soon as that half's copies finish (smaller pipeline tail)
                cuts = [nch * h // out_halves for h in range(out_halves + 1)]
                outs = [
                    out_pool.tile([HOP, B_BLK, cuts[h + 1] - cuts[h], C], out_dt,
                                  tag=f"out{h}", name=f"out{h}_{blk}")
                    for h in range(out_halves)
                ]
                for g in range(N_GRP):
                    o0 = g * C_GRP
                    ps = psum_pool.tile([HOP, C_GRP, B_BLK, nch], F32, tag="ps")
                    for j in range(C_GRP):
                        nc.tensor.matmul(
                            ps[:, j, :, :], bands_sb[:, o0 + j, :],
                            xh[:, :, :, o0 + j],
                            start=True, stop=True,
                        )
                    for h in range(out_halves):
                        n0, n1 = cuts[h], cuts[h + 1]
                        src = ps[:, :, :, n0:n1].rearrange("p j s n -> p s n j")
                        dst = outs[h][:, :, :, o0 : o0 + C_GRP]
                        if COPY_MODE == "dve" or g % 2 == 0:
                            nc.vector.tensor_copy(dst, src)
                        else:
                            nc.scalar.copy(dst, src)

                for h in range(out_halves):
                    n0, n1 = cuts[h], cuts[h + 1]
                    nf = min(n1, nfull) - n0
                    for s in range(B_BLK):
                        dstA = y_d[b0 + s, (lo + n0) * HOP : (lo + n0 + nf) * HOP,
                                   :].rearrange("(n p) c -> p n c", p=HOP)
                        out_eng.dma_start(dstA, outs[h][:, s, 0:nf, :],
                                          single_packet=SINGLE_PACKET)
                    if has_tail and n1 == nch:
                        out_eng.dma_start(
                            y_d[b0 : b0 + B_BLK, N_FULL * HOP : L_OUT, :].rearrange(
                                "s p c -> p s c"),
                            outs[h][LAST_FRESH_OFF : LAST_FRESH_OFF + LAST_FRESH,
                                    :, nf, :],
                            single_packet=SINGLE_PACKET,
                        )

            last_halves = int(os.environ.get("KERNEL_LAST_OUT_HALVES", "1"))
            for blk in range(n_blk):
                b0 = blk * B_BLK
                halves = last_halves if blk == n_blk - 1 else 1
                do_block(blk, b0, 0, N_CHUNKS, nc.sync, nc.scalar,
                         out_halves=halves)
    nc.finalize()
    return nc


def make_bands(kernels: np.ndarray) -> np.ndarray:
    """kernels [C, K] -> band tensor [WIN, C*HOP] with
    bands[p, o, m] = kernels[o, p - m] for 0 <= p-m < K."""
    bands = np.zeros((WIN, C, HOP), dtype=np.float32)
    m = np.arange(HOP)
    for k in range(K):
        bands[m + k, :, m] = kernels[:, k]          # [HOP, C] block per tap
    return bands.reshape(WIN, C * HOP)


def make_in_maps(x: np.ndarray, bands: np.ndarray) -> list:
    x = np.ascontiguousarray(x, dtype=np.float32)
    np_dt = mybir.dt.np(MM_DT)
    if x.dtype != np_dt:
        x = x.astype(np_dt)
        bands = bands.astype(np_dt)
    return [
        {"x": x[i * B_SHARD : (i + 1) * B_SHARD], "bands": bands}
        for i in range(N_CORES)
    ]


_NC_CACHE: dict = {}


def kernel(x: np.ndarray, attention_weights: np.ndarray,
           proj_w: np.ndarray, proj_b: np.ndarray) -> np.ndarray:
    x = np.asarray(x)
    attention_weights = np.asarray(attention_weights)
    proj_w = np.asarray(proj_w)
    proj_b = np.asarray(proj_b)
    kernels = (attention_weights.astype(np.float64) @ proj_w.T.astype(np.float64)
               + proj_b.astype(np.float64)).astype(np.float32)   # [B, K] == [C, K]
    bands = make_bands(kernels)

    if "nc" not in _NC_CACHE:
        _NC_CACHE["nc"] = build_nc()
    nc = _NC_CACHE["nc"]

    in_maps = make_in_maps(x, bands)
    res = run_bass_kernel_spmd(nc, in_maps, core_ids=list(range(N_CORES)))
    out = np.concatenate([r["y"] for r in res.results], axis=0)
    return np.ascontiguousarray(out.astype(np.float32))
